# revision 1
# baseline (speedup 1.0000x reference)
"""BasicYATBlock kernel for Trainium2 (Bass/Tile), data-parallel over batch on 8 cores.

Computes, per sample (stride=2 block, 128ch 56x56 -> 256ch 28x28):
    identity = conv1x1_s2(x, w_short)
    dot      = conv3x3_s2_p1(x, w_yat)
    patch_sq = conv3x3_s2_p1(x*x, ones)          (per-patch squared norm)
    yat      = dot^2 / (patch_sq + |w|^2 - 2 dot + EPS) * scale
    out      = conv3x3_s1_p1(yat, w_lin) + identity
scale = (sqrt(256)/log1p(256))**alpha is folded into w_lin on the host
(conv is linear), so the device kernel never sees alpha.

All convs are TensorE matmuls: K=ci (partitions), M=co, N=output pixels
(one PSUM bank of 14 output rows = 392 f32 per matmul, 9 taps accumulated).
Matmul operands are float32r (1 PE cycle/row vs 4 for fp32; ~2^-12 relative
rounding, measured end-to-end rel err ~1.6e-4 on hardware).

Layout choices (per core, everything lives in SBUF once loaded):
- x is stored zero-padded to 58x58 with even/odd columns deinterleaved
  ([row][parity][col/2]) so every stride-2 conv tap is a basic slice with
  unit inner stride; yat planes are zero-padded to 30x30 for conv2.
- x DMAs are contiguous (line rate), staged inside the xsq scratch tile,
  then pad-copied/deinterleaved on DVE.
- patch_sq: sample 0 uses all-ones matmuls per tap (keeps the PE busy
  during the cold head); samples 1-3 pre-reduce the 3x3 stencil per
  channel (adds on the lightly-loaded Pool engine) and use a single
  ones-matmul per chunk.
- software pipeline keeps the in-order PE queue fed: dots(s+1) and
  phase_b(s) are emitted between a sample's conv1 and its PSUM-consuming
  elementwise chain.
- cold-start insurance: a warmup matmul burst on the ones tile spans the
  idle head (keeps the PE p-state/HAM clock warm before the first real
  matmul), and a no-op square(1)=1 primes the ACT function table (~1.3us
  load) during dead time.
Cost-model time: ~86.1us/core (PE ~92% wall occupancy).
"""

import numpy as np

import concourse.bass as bass
import concourse.bacc as bacc
import concourse.mybir as mybir
from concourse import tile
from concourse.bass_utils import run_bass_kernel_spmd

F32 = mybir.dt.float32
F32R = mybir.dt.float32r

N_CORES = 8
NPER = 4          # samples per core
CI = 128          # input channels
CO = 256          # output channels (2 tiles of 128)
H = 56            # input spatial
XW = 58           # padded x plane width
OH = 28           # output spatial
CH = 14           # output rows per chunk
NCH = 2           # chunks per plane (2*14 = 28)
NPIX = CH * OH    # 392 free elements per matmul / PSUM tile
PW = 30           # padded yat plane width (28 + 2)
EPS = 0.007

POS_ORDER = [(kh, kw) for kh in range(3) for kw in range(3)]
WARMUP_MMS = 16   # PE ramp warmup matmuls during the idle head


def _x_tap(kh, kw, c):
    """Slice params into the padded-x (a hh b ww) layout for stride-2 tap
    (kh,kw) of output chunk c: padded input row = 2*oh + kh = 2*a + hh,
    col = 2*ow + kw = 2*b + ww."""
    a0 = c * CH + (1 if kh == 2 else 0)
    hh = kh % 2
    b0 = 1 if kw == 2 else 0
    ww = kw % 2
    return a0, hh, b0, ww


def build_nc(mm_dtype=F32R, nc=None, loop_n=1):
    if nc is None:
        nc = bass.Bass()

    x_d = nc.dram_tensor("x", [NPER, CI, H, H], mm_dtype, kind="ExternalInput")
    wyat_d = nc.dram_tensor("wyatT", [CI, 9, CO], mm_dtype, kind="ExternalInput")
    wlin_d = nc.dram_tensor("wlinT", [2, 128, 9, CO], mm_dtype, kind="ExternalInput")
    wshort_d = nc.dram_tensor("wshortT", [CI, CO], mm_dtype, kind="ExternalInput")
    wsqe_d = nc.dram_tensor("wsqe", [128, 2], F32, kind="ExternalInput")
    out_d = nc.dram_tensor("out", [NPER, CO, OH, OH], F32, kind="ExternalOutput")

    with tile.TileContext(nc) as tc:
        with (
            tc.tile_pool(name="const", bufs=1) as const,
            tc.tile_pool(name="xsqp", bufs=2) as xsqp,
            tc.tile_pool(name="scratch", bufs=2) as scratch,
            tc.tile_pool(name="stencil", bufs=2) as stencil,
            tc.tile_pool(name="outp", bufs=2) as outp,
            tc.tile_pool(name="psum", bufs=8, space="PSUM") as psum,
        ):
            wyat_sb = const.tile([CI, 9, CO], mm_dtype, tag="wyat")
            wlin_sb = const.tile([128, 2, 9, CO], mm_dtype, tag="wlin")
            wshort_sb = const.tile([CI, CO], mm_dtype, tag="wshort")
            wsqe_sb = const.tile([128, 2], F32, tag="wsqe")
            ones_sb = const.tile([CI, NPIX], mm_dtype, tag="ones")
            x_sb = [const.tile([CI, XW * XW], mm_dtype, tag=f"x{s}", name=f"x_sb{s}")
                    for s in range(NPER)]
            yat_sb = [const.tile([128, 2, PW * PW], mm_dtype, tag=f"yat{s}", name=f"yat_sb{s}")
                      for s in range(NPER)]

            def emit_iter(_it=0):
                XS = {}

                def load_x(s):
                    # padded plane stored col-DEINTERLEAVED: flat layout
                    # [row(58)][parity ww(2)][b(29)], padded col = 2b + ww,
                    # so every conv tap reads unit-stride along b.
                    x4 = x_sb[s][:].rearrange("p (r ww b) -> p r ww b",
                                              ww=2, b=XW // 2)
                    f4 = x4.bitcast(F32)
                    nc.gpsimd.memset(f4[:, 0, :, :], 0.0)          # row 0
                    nc.gpsimd.memset(f4[:, XW - 1, :, :], 0.0)     # row 57
                    nc.gpsimd.memset(f4[:, 1:XW - 1, 0, 0], 0.0)   # col 0
                    nc.gpsimd.memset(f4[:, 1:XW - 1, 1, 28], 0.0)  # col 57
                    # contiguous (line-rate) DMA staged inside this sample's xsq
                    # tile (whose contents are overwritten by the square later),
                    # then DVE deinterleaving pad-copies
                    xsq = xsqp.tile([CI, XW * XW], mm_dtype, tag="xsq",
                                    name=f"xsq{s}")
                    XS[s] = xsq

                    def pad_copies(r0, r1, stage_flat, nr):
                        st5 = stage_flat.rearrange(
                            "p (h b2 w2) -> p h b2 w2", b2=H // 2, w2=2)
                        # padded col 2b+1 <- true col 2b   (b = 0..27)
                        nc.vector.tensor_copy(
                            out=x4[:, r0:r1, 1, 0:H // 2],
                            in_=st5[:, :nr, :, 0])
                        # padded col 2b   <- true col 2b-1 (b = 1..28)
                        nc.vector.tensor_copy(
                            out=x4[:, r0:r1, 0, 1:H // 2 + 1],
                            in_=st5[:, :nr, :, 1])

                    if s == 0:
                        # split halves so chunk-0 matmuls start as soon as
                        # the top half lands
                        nc.sync.dma_start(out=xsq[:, :29 * H],
                                          in_=x_d[s, :, 0:29].rearrange(
                                              "c h w -> c (h w)"))
                        nc.sync.dma_start(out=xsq[:, 29 * H:H * H],
                                          in_=x_d[s, :, 29:H].rearrange(
                                              "c h w -> c (h w)"))
                        pad_copies(1, 30, xsq[:, :29 * H], 29)
                        pad_copies(30, 1 + H, xsq[:, 29 * H:H * H], 27)
                    else:
                        nc.sync.dma_start(out=xsq[:, :H * H],
                                          in_=x_d[s].rearrange("c h w -> c (h w)"))
                        pad_copies(1, 1 + H, xsq[:, :H * H], H)

                # x loads own the sync (HWDGE) queue; weights go via gpsimd
                # (SWDGE) so the first matmul's inputs arrive in parallel.
                nc.gpsimd.memset(ones_sb[:].bitcast(F32), 1.0)
                if _it == 0:
                    # prime the ACT function table (~1.3us load) during the
                    # idle head; square(1.0) == 1.0 so ones stays intact
                    nc.scalar.square(ones_sb[:, :1], ones_sb[:, :1])
                nc.gpsimd.dma_start(out=wyat_sb[:], in_=wyat_d[:])
                load_x(0)
                if _it == 0:
                    # warmup burst: keep the PE busy through its p-state /
                    # HAM ramp while the first x DMA is in flight, so the
                    # real matmuls run at full clock from the start
                    pw = psum.tile([128, NPIX], F32, tag="ps", name="pwarm")
                    for _w in range(WARMUP_MMS):
                        nc.tensor.matmul(pw[:], ones_sb[:, :128], ones_sb[:],
                                         start=True, stop=True)
                nc.gpsimd.dma_start(out=wsqe_sb[:], in_=wsqe_d[:])
                load_x(1)
                # yat borders only (the DVE writes cover the interior) so
                # the Pool queue stays clear for the stencil adds
                for s in range(NPER):
                    yb = yat_sb[s][:].rearrange(
                        "p t (r q) -> p t r q", q=PW).bitcast(F32)
                    nc.gpsimd.memset(yb[:, :, 0, :], 0.0)
                    nc.gpsimd.memset(yb[:, :, PW - 1, :], 0.0)
                    nc.gpsimd.memset(yb[:, :, 1:PW - 1, 0], 0.0)
                    nc.gpsimd.memset(yb[:, :, 1:PW - 1, PW - 1], 0.0)
                # wlin/wshort ride the sync queue (free after the early x
                # loads; needed only by phase_b) to keep SWDGE clear too
                for t in range(2):
                    nc.sync.dma_start(out=wlin_sb[:, t], in_=wlin_d[t])
                nc.sync.dma_start(out=wshort_sb[:], in_=wshort_d[:])
                load_x(2)
                load_x(3)

                A = {}  # per-sample state: x5, pt tiles, rsum

                def prep(s):
                    """x^2 square (+ DVE stencil pre-reduction for s>0)."""
                    x5 = x_sb[s][:].rearrange(
                        "p (a hh ww b) -> p a hh ww b", hh=2, ww=2, b=XW // 2
                    )
                    xsq = XS[s]
                    if s == 0:
                        nc.scalar.square(xsq[:, :30 * XW], x_sb[s][:, :30 * XW])
                        nc.scalar.square(xsq[:, 30 * XW:], x_sb[s][:, 30 * XW:])
                    else:
                        nc.scalar.square(xsq[:], x_sb[s][:])
                    st = {"x5": x5, "xsq": xsq}
                    if s != 0:
                        xq_c = xsq[:].rearrange("p (h ww b) -> p h ww b",
                                                ww=2, b=XW // 2)
                        csum = stencil.tile([128, XW * OH], F32, tag="csum",
                                            name=f"csum{s}")
                        rsum = stencil.tile([128, OH * OH], mm_dtype, tag="rsum",
                                            name=f"rsum{s}")
                        c3 = csum[:].rearrange("p (h w) -> p h w", w=OH)
                        # col3sum on the (lightly loaded) Pool engine to
                        # decongest the DVE queue
                        nc.gpsimd.tensor_add(
                            out=c3, in0=xq_c[:, :, 0, 0:OH], in1=xq_c[:, :, 1, 0:OH])
                        nc.gpsimd.tensor_add(
                            out=c3, in0=c3, in1=xq_c[:, :, 0, 1:OH + 1])
                        cs_r = csum[:].rearrange("p (a hh w) -> p a hh w",
                                                 hh=2, w=OH)
                        r3 = rsum[:].rearrange("p (h w) -> p h w", w=OH)
                        nc.gpsimd.tensor_add(
                            out=r3, in0=cs_r[:, 0:OH, 0, :], in1=cs_r[:, 0:OH, 1, :])
                        nc.gpsimd.tensor_add(
                            out=r3, in0=r3, in1=cs_r[:, 1:OH + 1, 0, :])
                        st["rsum"] = rsum
                    A[s] = st

                def dots(s):
                    """conv1 matmuls. Sample 0 also does patch_sq via taps
                    (keeps PE busy during the cold head); later samples get
                    patch_sq from the pre-reduced stencil, emitted in psq_yat
                    (after the next phase_b) so the in-order PE queue never
                    blocks on the DVE stencil chain."""
                    st = A[s]
                    x5, xsq = st["x5"], st["xsq"]
                    xq5 = xsq[:].rearrange(
                        "p (a hh ww b) -> p a hh ww b", hh=2, ww=2, b=XW // 2
                    )
                    pt = [[psum.tile([128, NPIX], F32, tag="ps", name=f"pA{s}_{c}_{j}")
                           for j in range(3)] for c in range(NCH)]
                    st["pt"] = pt
                    nj = 3 if s == 0 else 2
                    # sample 0: chunk-outer order so chunk-0 matmuls only wait
                    # for the top half of the plane
                    loop = ([(c, j) for c in range(NCH) for j in range(nj)]
                            if s == 0 else
                            [(c, j) for j in range(nj) for c in range(NCH)])
                    for c, j in loop:
                        for pi, (kh, kw) in enumerate(POS_ORDER):
                            if j < 2:
                                lhsT = wyat_sb[:, kh * 3 + kw, j * 128:(j + 1) * 128]
                            else:
                                lhsT = ones_sb[:, :128]
                            a0, hh, b0, ww = _x_tap(kh, kw, c)
                            src = x5 if j < 2 else xq5
                            rhs = src[:, a0:a0 + CH, hh, ww, b0:b0 + OH]
                            nc.tensor.matmul(
                                pt[c][j][:], lhsT, rhs,
                                start=(pi == 0), stop=(pi == 8),
                            )

                def psq_yat(s):
                    """patch_sq matmuls (s>0) + YAT elementwise -> yat_sb[s]."""
                    st = A[s]
                    pt = st["pt"]
                    if s != 0:
                        rsum = st["rsum"]
                        for c in range(NCH):
                            nc.tensor.matmul(
                                pt[c][2][:], ones_sb[:, :128],
                                rsum[:, c * NPIX:(c + 1) * NPIX],
                                start=True, stop=True,
                            )
                    y3 = yat_sb[s][:].rearrange("p t (r q) -> p t r q", q=PW)
                    for c in range(NCH):
                        p_psq = pt[c][2]
                        for t in range(2):
                            p_dot = pt[c][t]
                            psqe = scratch.tile([128, NPIX], F32, tag="psqe")
                            d = scratch.tile([128, NPIX], F32, tag="d")
                            r = scratch.tile([128, NPIX], F32, tag="r")
                            num = scratch.tile([128, NPIX], F32, tag="num")
                            # psqe = patch_sq + (|w|^2 + eps)
                            nc.scalar.activation(
                                psqe[:], p_psq[:],
                                mybir.ActivationFunctionType.Identity,
                                bias=wsqe_sb[:, t:t + 1], scale=1.0,
                            )
                            # d = -2*dot + psqe
                            nc.vector.scalar_tensor_tensor(
                                out=d[:], in0=p_dot[:], scalar=-2.0, in1=psqe[:],
                                op0=mybir.AluOpType.mult, op1=mybir.AluOpType.add,
                            )
                            nc.vector.reciprocal_approx_fast(out=r[:], in_=d[:])
                            nc.scalar.square(num[:], p_dot[:])
                            nc.vector.tensor_mul(
                                out=y3[:, t, c * CH + 1:c * CH + 1 + CH, 1:1 + OH],
                                in0=num[:].rearrange("p (r q) -> p r q", q=OH),
                                in1=r[:].rearrange("p (r q) -> p r q", q=OH),
                            )

                def phase_b(s):
                    """conv2 (3x3 s1 p1 on yat) + 1x1 s2 shortcut -> out."""
                    x5 = x_sb[s][:].rearrange(
                        "p (a hh ww b) -> p a hh ww b", hh=2, ww=2, b=XW // 2
                    )
                    y3 = yat_sb[s][:].rearrange("p t (r q) -> p t r q", q=PW)
                    for t in range(2):
                        out_t = outp.tile([128, 2 * NPIX], F32, tag="out")
                        for c in range(NCH):
                            p = psum.tile([128, NPIX], F32, tag="ps",
                                          name=f"pB{s}_{t}_{c}")
                            # 1x1 stride-2 shortcut: padded row 2*oh+1, col 2*ow+1
                            sc_rhs = x5[:, c * CH:(c + 1) * CH, 1, 1, 0:OH]
                            nc.tensor.matmul(
                                p[:], wshort_sb[:, t * 128:(t + 1) * 128],
                                sc_rhs, start=True, stop=False,
                            )
                            # kh-major order: kh<2 taps only read the
                            # first yat chunk of the rows they touch, so
                            # they unblock before the second chunk's DVE
                            # write lands
                            taps = [(kh, ci_t, kw) for kh in range(3)
                                    for ci_t in range(2) for kw in range(3)]
                            for ti, (kh, ci_t, kw) in enumerate(taps):
                                lhsT = wlin_sb[:, ci_t, kh * 3 + kw,
                                               t * 128:(t + 1) * 128]
                                rhs = y3[:, ci_t, c * CH + kh:c * CH + kh + CH,
                                         kw:kw + OH]
                                nc.tensor.matmul(
                                    p[:], lhsT, rhs,
                                    start=False, stop=(ti == len(taps) - 1),
                                )
                            nc.scalar.copy(out_t[:, c * NPIX:(c + 1) * NPIX], p[:])
                            if s == NPER - 1:
                                # last sample: per-chunk DMA so the final store
                                # doesn't wait for the second chunk's copy
                                nc.sync.dma_start(
                                    out=out_d[s, t * 128:(t + 1) * 128].rearrange(
                                        "c h w -> c (h w)")[:, c * NPIX:(c + 1) * NPIX],
                                    in_=out_t[:, c * NPIX:(c + 1) * NPIX],
                                )
                        if s != NPER - 1:
                            nc.sync.dma_start(
                                out=out_d[s, t * 128:(t + 1) * 128].rearrange(
                                    "c h w -> c (h w)"),
                                in_=out_t[:],
                            )

                # software pipeline: PE queue order is dots(0), dots(1),
                # B(0), psq+yat(1), dots(2), B(1), ... so the PE never waits
                # on the DVE stencil/yat chains of the in-flight sample.
                prep(0)
                dots(0)
                psq_yat(0)
                prep(1)
                dots(1)
                phase_b(0)
                psq_yat(1)
                prep(2)
                dots(2)
                phase_b(1)
                psq_yat(2)
                prep(3)
                dots(3)
                psq_yat(3)
                phase_b(2)
                phase_b(3)

            for _it in range(loop_n):
                emit_iter(_it)

    return nc


_NC_CACHE = {}


def _get_nc(mm_dtype=F32R, loop_n=1):
    key = (str(mm_dtype), loop_n)
    if key not in _NC_CACHE:
        nc = bacc.Bacc(None, target_bir_lowering=False)
        build_nc(mm_dtype, nc=nc, loop_n=loop_n)
        nc.compile()
        _NC_CACHE[key] = nc
    return _NC_CACHE[key]


def prep_weights(w_yat, alpha, w_lin, w_short):
    scale = float((np.sqrt(np.float32(CO)) / np.log1p(np.float32(CO))) ** np.float32(alpha[0]))
    wyatT = np.ascontiguousarray(
        w_yat.astype(np.float32).transpose(1, 2, 3, 0)).reshape(CI, 9, CO)
    wlinT = np.ascontiguousarray(
        (w_lin.astype(np.float32) * np.float32(scale)).transpose(1, 2, 3, 0)
    ).reshape(2, 128, 9, CO)
    wshortT = np.ascontiguousarray(
        w_short.astype(np.float32)[:, :, 0, 0].transpose(1, 0))
    wsq = (w_yat.astype(np.float32) ** 2).sum(axis=(1, 2, 3))
    wsqe = np.ascontiguousarray((wsq + np.float32(EPS)).reshape(2, 128).T)
    return wyatT, wlinT, wshortT, wsqe


def bench(x, w_yat, alpha, w_lin, w_short, iters=20, _mm_dtype=F32R,
          loop_n=1):
    """Time the 8-core PJRT executable on device-resident inputs.

    Returns (min_wall_ns_per_iter, outputs) — wall time includes axon
    dispatch overhead, so it is an upper bound on device exec time.
    """
    import time as _time

    import jax
    import jax.numpy as jnp
    from jax.sharding import Mesh, NamedSharding, PartitionSpec
    from jax.experimental.shard_map import shard_map

    from concourse import bass2jax as b2j

    b2j.install_neuronx_cc_hook()
    nc = _get_nc(_mm_dtype, loop_n=loop_n)

    x = np.ascontiguousarray(np.asarray(x, dtype=np.float32))
    wyatT, wlinT, wshortT, wsqe = prep_weights(
        np.asarray(w_yat), np.asarray(alpha), np.asarray(w_lin),
        np.asarray(w_short))
    per_core_vals = {"wyatT": wyatT, "wlinT": wlinT, "wshortT": wshortT,
                     "wsqe": wsqe}

    import concourse.mybir as _mybir
    partition_name0 = (nc.partition_id_tensor.name
                       if nc.partition_id_tensor else None)
    in_names, out_names, out_avals = [], [], []
    for alloc in nc.m.functions[0].allocations:
        if not isinstance(alloc, _mybir.MemoryLocationSet):
            continue
        name = alloc.memorylocations[0].name
        if alloc.kind == "ExternalInput":
            if name == partition_name0:
                continue
            in_names.append(name)
        elif alloc.kind == "ExternalOutput":
            out_names.append(name)
            out_avals.append(jax.core.ShapedArray(
                tuple(alloc.tensor_shape), _mybir.dt.np(alloc.dtype)))
    n_params = len(in_names)
    all_in_names = in_names + out_names

    partition_name = (nc.partition_id_tensor.name
                      if nc.partition_id_tensor else None)
    if partition_name is not None:
        all_in_names.append(partition_name)

    def _call(args):
        operands = list(args)
        if partition_name is not None:
            operands.append(b2j.partition_id_tensor())
        return b2j._bass_exec_p.bind(
            *operands,
            out_avals=tuple(out_avals),
            in_names=tuple(all_in_names),
            out_names=tuple(out_names),
            lowering_input_output_aliases=(),
            sim_require_finite=True,
            sim_require_nnan=True,
            nc=nc,
        )

    def _body(*args):
        return tuple(_call(args))

    devices = jax.devices()[:N_CORES]
    mesh = Mesh(np.asarray(devices), ("core",))
    spec = PartitionSpec("core")
    donate = tuple(range(n_params, n_params + len(out_names)))
    sharded = jax.jit(
        shard_map(_body, mesh=mesh, in_specs=(spec,) * (n_params + len(out_names)),
                  out_specs=(spec,) * len(out_names), check_rep=False),
        donate_argnums=donate, keep_unused=True)

    concat_in = []
    for name in in_names:
        if name == "x":
            concat_in.append(x)
        else:
            v = per_core_vals[name]
            concat_in.append(np.concatenate([v] * N_CORES, axis=0))
    dev_in = [jax.device_put(a, NamedSharding(mesh, spec)) for a in concat_in]

    zero_shapes = [(N_CORES * av.shape[0], *av.shape[1:]) for av in out_avals]
    make_zeros = jax.jit(
        lambda: tuple(jnp.zeros(s, dtype=av.dtype)
                      for s, av in zip(zero_shapes, out_avals)),
        out_shardings=tuple(NamedSharding(mesh, spec) for _ in out_avals))
    zs = make_zeros()
    jax.block_until_ready(zs)

    # correctness output from the single-call program
    outs = sharded(*dev_in, *make_zeros())
    jax.block_until_ready(outs)
    out_np = np.asarray(outs[0]).reshape(N_CORES, *out_avals[0].shape)
    full = out_np.reshape(N_CORES * NPER, CO, OH, OH)

    # slope timing: dispatch k independent executions asynchronously and
    # block once — the device serializes them, so T(k2)-T(k1) isolates the
    # per-execution device time from the axon dispatch overhead
    def timed(k, reps):
        ts = []
        for _ in range(reps):
            zss = [make_zeros() for _ in range(k)]
            jax.block_until_ready(zss)
            t0 = _time.perf_counter()
            rs = [sharded(*dev_in, *zs) for zs in zss]
            jax.block_until_ready(rs)
            ts.append(_time.perf_counter() - t0)
        return min(ts)

    k1, k2 = 1, 13
    timed(k1, 2)  # warm
    t1 = timed(k1, iters)
    t2 = timed(k2, max(3, iters // 3))
    per_exec_ns = int((t2 - t1) / (k2 - k1) * 1e9)
    return per_exec_ns, full, (t1, t2)


def kernel(x, w_yat, alpha, w_lin, w_short, _mm_dtype=F32R, _trace=False):
    import os
    # this axon deployment has no NTFF hook (antenv.axon_hooks absent);
    # make sure an inherited BASS_TRACE can't route us into that path
    if not _trace:
        os.environ["BASS_NEVER_TRACE"] = "1"
    x = np.ascontiguousarray(np.asarray(x, dtype=np.float32))
    wyatT, wlinT, wshortT, wsqe = prep_weights(
        np.asarray(w_yat), np.asarray(alpha), np.asarray(w_lin),
        np.asarray(w_short))
    nc = _get_nc(_mm_dtype)
    in_maps = []
    for i in range(N_CORES):
        in_maps.append({
            "x": x[i * NPER:(i + 1) * NPER],
            "wyatT": wyatT, "wlinT": wlinT, "wshortT": wshortT, "wsqe": wsqe,
        })
    res = run_bass_kernel_spmd(nc, in_maps, core_ids=list(range(N_CORES)),
                               trace=_trace)
    out = np.concatenate([res.results[i]["out"] for i in range(N_CORES)], axis=0)
    if _trace:
        kernel.last_results = res
    return out



# revision 6
# speedup vs baseline: 1.9508x; 1.9508x over previous
"""BasicYATBlock kernel for Trainium2 (Bass/Tile), data-parallel over batch on 8 cores.

Computes, per sample (stride=2 block, 128ch 56x56 -> 256ch 28x28):
    identity = conv1x1_s2(x, w_short)
    dot      = conv3x3_s2_p1(x, w_yat)
    patch_sq = conv3x3_s2_p1(x*x, ones)          (per-patch squared norm)
    yat      = dot^2 / (patch_sq + |w|^2 - 2 dot + EPS) * scale
    out      = conv3x3_s1_p1(yat, w_lin) + identity

All three convs are TensorE matmuls. The two big convs run in fp8 with
DoubleRow perf mode (2 K-tiles per instruction at 0.5 PE cycles/row).
DoubleRow ifmaps must be [p][2][N] access patterns, so layouts are built
to make every tap window one contiguous 392-element run:
  - x is uploaded as six 29x28 parity sub-planes (row parity hh x three
    column alignments kw=0/1/2, zero-padding baked in on the host), so a
    stride-2 3x3 tap is rows a0..a0+13 x all 28 cols = flat [a0*28, +392).
    conv1 pairs 2 taps per matmul (pair 5 = tap9 + zero weights).
  - patch_sq: same tap pairs over xsq=(2x)^2 (ACT square of the fp8
    planes), ones lhsT; |w|^2+eps rides in as a K=1 DoubleRow matmul
    against an all-ones rhs, so no separate bias op is needed.
  - conv2 DoubleRow-pairs the two ci tiles (K=256 per instruction) on a
    31x28 row-padded yat plane; kw!=1 taps read one column out of line,
    which wraps to the adjacent row's edge column — those spurious terms
    are cancelled by 6 tiny N=14 correction matmuls with negated weights.
The identity shortcut stays bf16 (it dominates the output magnitude;
fp8 there would eat most of the 2e-2 error budget). Scale bookkeeping:
x8=e4m3(x), wyat8=e4m3(16 w), xsq=e4m3(4 x^2), wsq8=e4m3(4(|w|^2+eps)),
so d4 = (pt * -0.5) + pp = 4(dist^2+eps); num = pt^2 = 256 dot^2;
yat8 = e4m3(num/d4) = e4m3(64 yat); wlin8 = e5m2(w_lin*alpha_scale/64).
The 1/64 keeps wlin out of fp8 subnormals (e5m2 reaches 6e-5) and the
64 keeps yat8 in e4m3 normal range.

Elementwise work is spread so no engine exceeds the PE: ACT does the x
squares + t0 output copies, DVE does d4/recip/num, Pool (gpsimd) does
the yat multiply + t1 output copies.
"""

import numpy as np
import ml_dtypes

import bass_rust
import concourse.bass as bass
import concourse.bacc as bacc
import concourse.mybir as mybir
from concourse import tile
from concourse.bass_utils import run_bass_kernel_spmd

F32 = mybir.dt.float32
BF16 = mybir.dt.bfloat16
F8E4 = mybir.dt.float8e4
F8E5 = mybir.dt.float8e5

E4 = ml_dtypes.float8_e4m3
E5 = ml_dtypes.float8_e5m2
BF = ml_dtypes.bfloat16

N_CORES = 8
NPER = 4          # samples per core
CI = 128          # input channels
CO = 256          # output channels (2 tiles of 128)
H = 56            # input spatial
OH = 28           # output spatial
CH = 14           # output rows per chunk
NCH = 2           # chunks per plane
NPIX = CH * OH    # 392 free elements per PSUM tile
PBANK = NPIX + 2 * CH  # psum slot: 392 out + 2x14 wrap-correction scratch
PLANE = 29 * OH   # 812: one parity sub-plane
XSZ = 6 * PLANE   # 4872: six sub-planes per sample
YROWS = 32        # yat plane rows (2 top pad + 28 + 2 bottom pad)
YSZ = YROWS * OH  # 896
EPS = 0.007

# conv1/patch_sq tap pairs (tap index = kh*3+kw); pair 5 = (tap 8, dummy)
TAP_PAIRS = [(0, 1), (2, 3), (4, 5), (6, 7), (8, None)]
# conv2 correction taps: kw=0 group then kw=2 group
COR_TAPS = [0, 3, 6, 2, 5, 8]
WARMUP_MMS = 10
DR = mybir.MatmulPerfMode.DoubleRow


def _xoff(kh, kw, c):
    """Flat offset of stride-2 tap (kh,kw), chunk c in the 6-sub-plane
    x layout: plane (kh%2)*3+kw, rows a0..a0+13, all 28 cols."""
    return ((kh % 2) * 3 + kw) * PLANE + (c * CH + (1 if kh == 2 else 0)) * OH


def _rhs3(flat_ap, off, part_dim, pair_stride, n, nstride=1):
    """3-dim DoubleRow rhs AP [p][pair 2][n]."""
    d = flat_ap.copy()
    d.ap = bass_rust.VecI64Pair([list(part_dim), [pair_stride, 2],
                                 [nstride, n]])
    d.offset = flat_ap.offset + off
    return d


def build_nc(nc=None, loop_n=1):
    if nc is None:
        nc = bass.Bass()

    x8_d = nc.dram_tensor("x8", [NPER, CI, XSZ], F8E4, kind="ExternalInput")
    xq_d = nc.dram_tensor("xq", [NPER, CI, OH * OH], BF16, kind="ExternalInput")
    wyat_d = nc.dram_tensor("wyat8", [CI, 5, 2, CO], F8E4, kind="ExternalInput")
    wlin_d = nc.dram_tensor("wlin8", [CI, 2, 9, CO], F8E5, kind="ExternalInput")
    wshort_d = nc.dram_tensor("wshort", [CI, CO], BF16, kind="ExternalInput")
    wsq_d = nc.dram_tensor("wsq8", [1, 2, CO], F8E4, kind="ExternalInput")
    out_d = nc.dram_tensor("out", [NPER, CO, OH, OH], F32, kind="ExternalOutput")

    with tile.TileContext(nc) as tc:
        with (
            tc.tile_pool(name="const", bufs=1) as const,
            tc.tile_pool(name="xsqp", bufs=2) as xsqp,
            tc.tile_pool(name="scr", bufs=3) as scr,
            tc.tile_pool(name="outp", bufs=2) as outp,
            tc.tile_pool(name="psum", bufs=8, space="PSUM") as psum,
        ):
            wyat_sb = const.tile([CI, 5, 2, CO], F8E4, tag="wyat")
            wlin_sb = const.tile([CI, 2, 9, CO], F8E5, tag="wlin")
            wshort_sb = const.tile([CI, CO], BF16, tag="wshort")
            wsq_sb = const.tile([1, 2, CO], F8E4, tag="wsq")
            ones_sb = const.tile([CI, 2, CI], F8E4, tag="ones")
            onez_sb = const.tile([CI, 2, CI], F8E4, tag="onez")
            ones1_sb = const.tile([1, 2 * NPIX], F8E4, tag="ones1")
            x8_sb = [const.tile([CI, XSZ], F8E4, tag=f"x{s}", name=f"x8_{s}")
                     for s in range(NPER)]
            xq_sb = [const.tile([CI, OH * OH], BF16, tag=f"xq{s}", name=f"xq_{s}")
                     for s in range(NPER)]
            yat_sb = [const.tile([CI, 2, YSZ], F8E4, tag=f"yat{s}",
                                 name=f"yat_{s}") for s in range(NPER)]

            def emit_iter(_it=0):
                XS = {}   # per-sample xsq tile
                PT = {}   # per-sample dot psums [c][t]
                PP = {}   # per-sample patch_sq psums [c][t]

                def load_x(s):
                    nc.sync.dma_start(out=x8_sb[s][:], in_=x8_d[s])
                    nc.sync.dma_start(out=xq_sb[s][:], in_=xq_d[s])

                # constants / weights: Pool (SWDGE) queue so they overlap
                # the sync-queue x loads
                nc.vector.memset(ones_sb[:], 1.0)
                nc.vector.memset(onez_sb[:, 0], 1.0)
                nc.vector.memset(onez_sb[:, 1], 0.0)
                nc.vector.memset(ones1_sb[:], 1.0)
                # yat plane pad rows (top 2, bottom 1)
                for s in range(NPER):
                    for t in range(2):
                        nc.vector.memset(yat_sb[s][:, t, :2 * OH], 0.0)
                        nc.vector.memset(yat_sb[s][:, t, 30 * OH:YSZ], 0.0)
                if _it == 0:
                    # prime the ACT Square table (~1.3us) during the idle
                    # head; square(1)=1 keeps the ones tile intact
                    nc.scalar.square(ones1_sb[:, :1], ones1_sb[:, :1])
                nc.gpsimd.dma_start(out=wyat_sb[:], in_=wyat_d[:])
                nc.gpsimd.dma_start(out=wsq_sb[:], in_=wsq_d[:])
                load_x(0)
                if _it == 0:
                    # keep the PE p-state ramp warm through the DMA head
                    pw = psum.tile([CI, PBANK], F32, tag="ps", name="pwarm")
                    ones_flat = ones_sb[:].rearrange("p a b -> p (a b)")
                    for _w in range(WARMUP_MMS):
                        nc.tensor.matmul(pw[:, :2 * CI], ones_sb[:, 0], ones_flat,
                                         start=True, stop=True)
                nc.gpsimd.dma_start(out=wlin_sb[:], in_=wlin_d[:])
                nc.gpsimd.dma_start(out=wshort_sb[:], in_=wshort_d[:])
                load_x(1)
                load_x(2)
                load_x(3)

                def prep(s):
                    """xsq = (2*x)^2 in e4m3 (ACT). Padding stays 0."""
                    xsq = xsqp.tile([CI, XSZ], F8E4, tag="xsq",
                                    name=f"xsq{s}")
                    XS[s] = xsq
                    sq = mybir.ActivationFunctionType.Square
                    if s == 0:
                        half = 3 * PLANE
                        nc.scalar.activation(xsq[:, :half], x8_sb[s][:, :half],
                                             sq, scale=2.0)
                        nc.scalar.activation(xsq[:, half:], x8_sb[s][:, half:],
                                             sq, scale=2.0)
                    else:
                        nc.scalar.activation(xsq[:], x8_sb[s][:], sq, scale=2.0)

                def dp(s):
                    """conv1 dot + patch_sq matmuls, per (chunk, co-tile)."""
                    xflat = x8_sb[s][:]
                    part = xflat.ap[0]
                    pt = [[None, None], [None, None]]
                    pp = [[None, None], [None, None]]
                    PT[s], PP[s] = pt, pp
                    ones1_k = ones1_sb[:].rearrange("p (k n) -> p k n", k=2)
                    for c in range(NCH):
                        for t in range(2):
                            ptile = psum.tile([CI, PBANK], F32, tag="ps",
                                              name=f"pt{s}_{c}_{t}")
                            pptile = psum.tile([CI, PBANK], F32, tag="ps",
                                               name=f"pp{s}_{c}_{t}")
                            pt[c][t], pp[c][t] = ptile, pptile
                            for src, dst, wsel in ((xflat, ptile, 0),
                                                   (XS[s][:], pptile, 1)):
                                spart = src.ap[0]
                                for pi, (ta, tb) in enumerate(TAP_PAIRS):
                                    oa = _xoff(ta // 3, ta % 3, c)
                                    ob = (oa + 1 if tb is None
                                          else _xoff(tb // 3, tb % 3, c))
                                    rhs = _rhs3(src, oa, spart, ob - oa, NPIX)
                                    if wsel == 0:
                                        lhsT = wyat_sb[:, pi, :,
                                                       t * CI:(t + 1) * CI]
                                    else:
                                        lhsT = (onez_sb if tb is None
                                                else ones_sb)[:]
                                    nc.tensor.matmul(
                                        dst[:, :NPIX], lhsT, rhs, perf_mode=DR,
                                        start=(pi == 0),
                                        stop=(pi == 4 and wsel == 0))
                            # |w|^2 + eps as K=1 DoubleRow vs all-ones rhs
                            nc.tensor.matmul(
                                pptile[:, :NPIX], wsq_sb[:, :, t * CI:(t + 1) * CI],
                                ones1_k, perf_mode=DR,
                                start=False, stop=True)

                def chain(s):
                    """YAT elementwise: d4 -> 1/d4 -> num -> yat8."""
                    pt, pp = PT[s], PP[s]
                    for c in range(NCH):
                        for t in range(2):
                            d4 = scr.tile([CI, NPIX], F32, tag="d")
                            r4 = scr.tile([CI, NPIX], F32, tag="r")
                            num = scr.tile([CI, NPIX], F32, tag="n")
                            nc.vector.scalar_tensor_tensor(
                                out=d4[:], in0=pt[c][t][:, :NPIX], scalar=-0.5,
                                in1=pp[c][t][:, :NPIX],
                                op0=mybir.AluOpType.mult,
                                op1=mybir.AluOpType.add)
                            nc.vector.reciprocal_approx_fast(out=r4[:],
                                                             in_=d4[:])
                            nc.vector.tensor_mul(out=num[:], in0=pt[c][t][:, :NPIX],
                                                 in1=pt[c][t][:, :NPIX])
                            nc.gpsimd.tensor_mul(
                                out=yat_sb[s][:, t,
                                              2 * OH + c * NPIX:
                                              2 * OH + (c + 1) * NPIX],
                                in0=num[:], in1=r4[:])

                def phase_b(s):
                    """conv2 (3x3 s1 p1, fp8 DR over ci pairs, wrap-corrected)
                    + bf16 1x1 s2 shortcut -> out."""
                    yflat = yat_sb[s][:].rearrange("p a b -> p (a b)")
                    ypart = yflat.ap[0]
                    for t in range(2):
                        out_t = outp.tile([CI, 2 * NPIX], F32, tag="out")
                        for c in range(NCH):
                            po = psum.tile([CI, PBANK], F32, tag="ps",
                                           name=f"po{s}_{t}_{c}")
                            po3 = po[:, :NPIX].rearrange(
                                "p (r q) -> p r q", q=OH)
                            nc.tensor.matmul(
                                po[:, :NPIX], wshort_sb[:, t * CI:(t + 1) * CI],
                                xq_sb[s][:, c * NPIX:(c + 1) * NPIX],
                                start=True, stop=False)
                            for ti in range(9):
                                kh, kw = ti // 3, ti % 3
                                off = (c * CH + kh + 1) * OH + kw - 1
                                rhs = _rhs3(yflat, off, ypart, YSZ, NPIX)
                                lhsT = wlin_sb[:, :, ti, t * CI:(t + 1) * CI]
                                nc.tensor.matmul(
                                    po[:, :NPIX], lhsT, rhs,
                                    perf_mode=DR, start=False, stop=False)
                            # accumulate the column-wrap spurious terms into
                            # the same bank's scratch region (contiguous psum
                            # writes only), then subtract into the edge cols
                            for ci_, ti in enumerate(COR_TAPS):
                                kh, kw = ti // 3, ti % 3
                                if kw == 0:
                                    off = (c * CH + kh) * OH + OH - 1
                                    oview = po[:, NPIX:NPIX + CH]
                                else:
                                    off = (c * CH + kh + 2) * OH
                                    oview = po[:, NPIX + CH:NPIX + 2 * CH]
                                rhs = _rhs3(yflat, off, ypart, YSZ, CH,
                                            nstride=OH)
                                lhsT = wlin_sb[:, :, ti,
                                               t * CI:(t + 1) * CI]
                                nc.tensor.matmul(
                                    oview, lhsT, rhs, perf_mode=DR,
                                    start=False, stop=(ci_ == 5))
                            eng = nc.vector if t == 0 else nc.gpsimd
                            sp0 = po[:, NPIX:NPIX + CH].rearrange(
                                "p (a b) -> p a b", b=1)
                            sp1 = po[:, NPIX + CH:NPIX + 2 * CH].rearrange(
                                "p (a b) -> p a b", b=1)
                            eng.tensor_sub(out=po3[:, :, 0:1],
                                           in0=po3[:, :, 0:1], in1=sp0)
                            eng.tensor_sub(out=po3[:, :, OH - 1:OH],
                                           in0=po3[:, :, OH - 1:OH], in1=sp1)
                            dst = out_t[:, c * NPIX:(c + 1) * NPIX]
                            if t == 0:
                                nc.scalar.copy(dst, po[:, :NPIX])
                            else:
                                nc.gpsimd.tensor_copy(out=dst, in_=po[:, :NPIX])
                            if s == NPER - 1:
                                nc.sync.dma_start(
                                    out=out_d[s, t * CI:(t + 1) * CI].rearrange(
                                        "c h w -> c (h w)")[:, c * NPIX:(c + 1) * NPIX],
                                    in_=dst)
                        if s != NPER - 1:
                            nc.sync.dma_start(
                                out=out_d[s, t * CI:(t + 1) * CI].rearrange(
                                    "c h w -> c (h w)"),
                                in_=out_t[:])

                # software pipeline: PE order dp0,dp1,B0,dp2,B1,dp3,B2,B3;
                # ACT order sq0,sq1,sq2,cp0,sq3,cp1,cp2,cp3 (squares are on
                # the PE-feed path and must not queue behind output copies)
                prep(0)
                dp(0)
                chain(0)
                prep(1)
                dp(1)
                chain(1)
                prep(2)
                phase_b(0)
                dp(2)
                chain(2)
                prep(3)
                phase_b(1)
                dp(3)
                chain(3)
                phase_b(2)
                phase_b(3)

            for _it in range(loop_n):
                emit_iter(_it)

    return nc


_NC_CACHE = {}


def _get_nc(loop_n=1):
    key = loop_n
    if key not in _NC_CACHE:
        nc = bacc.Bacc(None, target_bir_lowering=False)
        build_nc(nc=nc, loop_n=loop_n)
        nc.compile()
        _NC_CACHE[key] = nc
    return _NC_CACHE[key]


def prep_inputs(x, w_yat, alpha, w_lin, w_short):
    """Host-side dtype/layout prep for the full batch."""
    x = np.asarray(x, np.float32)
    w_yat = np.asarray(w_yat, np.float32)
    w_lin = np.asarray(w_lin, np.float32)
    w_short = np.asarray(w_short, np.float32)
    n = x.shape[0]

    # six 29x28 parity sub-planes with padding baked in:
    # plane (hh, v): rows = padded rows hh,hh+2,..,hh+56; cols per v:
    # v=0: padded cols 0,2,..,54; v=1: 1,3,..,55; v=2: 2,4,..,56
    xpad = np.zeros((n, CI, 58, 58), np.float32)
    xpad[:, :, 1:H + 1, 1:H + 1] = x
    planes = np.empty((n, CI, 6, 29, OH), np.float32)
    for hh in range(2):
        rows = xpad[:, :, hh:hh + 58:2, :]
        planes[:, :, hh * 3 + 0] = rows[:, :, :, 0:56:2]
        planes[:, :, hh * 3 + 1] = rows[:, :, :, 1:57:2]
        planes[:, :, hh * 3 + 2] = rows[:, :, :, 2:58:2]
    x8 = planes.astype(E4).reshape(n, CI, XSZ)
    xq = np.ascontiguousarray(x[:, :, ::2, ::2]).astype(BF).reshape(
        n, CI, OH * OH)

    # conv1 weights: x16, tap-paired [ci, pair, 2, co]
    wt = np.ascontiguousarray(
        (w_yat * np.float32(16.0)).transpose(1, 2, 3, 0)).reshape(CI, 9, CO)
    wyat8 = np.zeros((CI, 5, 2, CO), E4)
    wyat8[:, :4] = wt[:, :8].reshape(CI, 4, 2, CO).astype(E4)
    wyat8[:, 4, 0] = wt[:, 8].astype(E4)

    scale = float((np.sqrt(np.float32(CO)) / np.log1p(np.float32(CO)))
                  ** np.float32(np.asarray(alpha).ravel()[0]))
    wlin_t = np.ascontiguousarray(
        (w_lin * np.float32(scale / 64.0)).transpose(1, 2, 3, 0)
    ).reshape(2, CI, 9, CO).transpose(1, 0, 2, 3)
    wlin_t = np.ascontiguousarray(wlin_t)           # [ci, ci_tile, tap, co]
    wlin8 = wlin_t.astype(E5)

    wshort = np.ascontiguousarray(
        w_short[:, :, 0, 0].transpose(1, 0)).astype(BF)

    wsq = (w_yat ** 2).sum(axis=(1, 2, 3))
    wsq8 = np.zeros((1, 2, CO), E4)
    wsq8[0, 0] = (np.float32(4.0) * (wsq + np.float32(EPS))).astype(E4)

    return {"x8": x8, "xq": xq, "wyat8": wyat8, "wlin8": wlin8,
            "wshort": wshort, "wsq8": wsq8}


def kernel(x, w_yat, alpha, w_lin, w_short, _trace=False):
    import os
    # this axon deployment has no NTFF hook (antenv.axon_hooks absent);
    # make sure an inherited BASS_TRACE can't route us into that path
    if not _trace:
        os.environ["BASS_NEVER_TRACE"] = "1"
    full = prep_inputs(x, w_yat, alpha, w_lin, w_short)
    nc = _get_nc()
    in_maps = []
    for i in range(N_CORES):
        m = {k: v for k, v in full.items() if k not in ("x8", "xq")}
        m["x8"] = full["x8"][i * NPER:(i + 1) * NPER]
        m["xq"] = full["xq"][i * NPER:(i + 1) * NPER]
        in_maps.append(m)
    res = run_bass_kernel_spmd(nc, in_maps, core_ids=list(range(N_CORES)),
                               trace=_trace)
    out = np.concatenate([res.results[i]["out"] for i in range(N_CORES)],
                         axis=0)
    if _trace:
        kernel.last_results = res
    return out


# revision 12
# speedup vs baseline: 2.3023x; 1.1802x over previous
"""BasicYATBlock kernel for Trainium2 (Bass/Tile), data-parallel over batch on 8 cores.

Computes, per sample (stride=2 block, 128ch 56x56 -> 256ch 28x28):
    identity = conv1x1_s2(x, w_short)
    dot      = conv3x3_s2_p1(x, w_yat)
    patch_sq = conv3x3_s2_p1(x*x, ones)          (per-patch squared norm)
    yat      = dot^2 / (patch_sq + |w|^2 - 2 dot + EPS) * scale
    out      = conv3x3_s1_p1(yat, w_lin) + identity

All three convs are TensorE matmuls. The two big convs run in fp8 with
DoubleRow perf mode (2 K-tiles per instruction at 0.5 PE cycles/row).
DoubleRow ifmaps must be [p][2][N] access patterns, so layouts are built
to make every tap window one contiguous 392-element run:
  - x is uploaded as six 29x28 parity sub-planes (row parity hh x three
    column alignments kw=0/1/2, zero padding baked in on the host), row-
    split into a rows 0..15 half and a rows 14..28 half so the two output
    chunks' squares can start independently. A stride-2 3x3 tap is then
    rows a0..a0+13 x all 28 cols = one flat [off, off+392) run.
    conv1 pairs 2 taps per matmul (pair 5 = tap9 + zero weights).
  - patch_sq: same tap pairs over xsq = x^2 (squared fp8 planes), shared
    across both co-tiles; |w|^2+eps is added per co-tile by a cheap Pool
    tensor_scalar into SBUF (psqe) before the DVE chain.
  - conv2 DoubleRow-pairs the two ci tiles (K=256 per instruction) on a
    32x28 row-padded yat plane; kw!=1 taps read one column out of line,
    which wraps to the adjacent row's edge column — those spurious terms
    are accumulated by 6 tiny N=14 correction matmuls into scratch at the
    end of the same PSUM bank and subtracted from the edge columns.
The identity shortcut stays bf16 (it dominates the output magnitude;
fp8 there would eat most of the 2e-2 error budget). Scale bookkeeping:
x8=e4m3(x), wyat8=e4m3(16 w), xsq=e4m3(x^2), wsqe=f32(|w|^2+eps),
d = (pt * -1/8) + psqe = dist^2+eps; num = pt^2 = 256 dot^2;
yat8 = e4m3(256 yat); wlin8 = e5m2(w_lin*alpha_scale/256).
The 1/256 keeps wlin roughly in e5m2 normal range (reaches 6e-5) and
the 256 keeps yat8 in e4m3 normal range.

Elementwise work is spread so no engine exceeds the PE: ACT does the
squares (chunk-1 half of sample 0 goes to Pool so both chunks' patch_sq
can start early) + t0 output copies, DVE does d/recip, Pool (gpsimd)
does psqe/num/yat multiplies + t1 output copies. Output rides per-sample
[ci][co-tile][pix] DMAs (host transposes back); the last sample's DMAs
are split per chunk and spread across queues to shorten the tail.
"""

import numpy as np
import ml_dtypes

import bass_rust
import concourse.bass as bass
import concourse.bacc as bacc
import concourse.mybir as mybir
from concourse import tile
from concourse.bass_utils import run_bass_kernel_spmd

F32 = mybir.dt.float32
BF16 = mybir.dt.bfloat16
F8E4 = mybir.dt.float8e4
F8E5 = mybir.dt.float8e5

E4 = ml_dtypes.float8_e4m3
E5 = ml_dtypes.float8_e5m2
BF = ml_dtypes.bfloat16

N_CORES = 8
NPER = 4          # samples per core
CI = 128          # input channels
CO = 256          # output channels (2 tiles of 128)
H = 56            # input spatial
OH = 28           # output spatial
CH = 14           # output rows per chunk
NCH = 2           # chunks per plane
NPIX = CH * OH    # 392 free elements per PSUM tile
PBANK = NPIX + 2 * CH  # psum slot: 392 out + 2x14 wrap-correction scratch
HALF0 = 6 * 16 * OH    # 2688: rows 0..15 of the six sub-planes
HALF1 = 6 * 15 * OH    # 2520: rows 14..28
XSZ = HALF0 + HALF1    # 5208 per sample
YROWS = 32        # yat plane rows (2 top pad + 28 + 2 bottom pad)
YSZ = YROWS * OH  # 896
EPS = 0.007

# conv1/patch_sq tap pairs (tap index = kh*3+kw); pair 5 = (tap 8, dummy)
TAP_PAIRS = [(0, 1), (2, 3), (4, 5), (6, 7), (8, None)]
# conv2 correction taps: kw=0 group then kw=2 group
COR_TAPS = [0, 3, 6, 2, 5, 8]
WARMUP_MMS = 10
DR = mybir.MatmulPerfMode.DoubleRow


def _xoff(kh, kw, c):
    """Flat offset of stride-2 tap (kh,kw), chunk c in the half-split
    6-sub-plane x layout."""
    pl = (kh % 2) * 3 + kw
    a0 = c * CH + (1 if kh == 2 else 0)
    if c == 0:
        return pl * (16 * OH) + a0 * OH
    return HALF0 + pl * (15 * OH) + (a0 - CH) * OH


def _rhs3(flat_ap, off, part_dim, pair_stride, n, nstride=1):
    """3-dim DoubleRow rhs AP [p][pair 2][n]."""
    d = flat_ap.copy()
    d.ap = bass_rust.VecI64Pair([list(part_dim), [pair_stride, 2],
                                 [nstride, n]])
    d.offset = flat_ap.offset + off
    return d


def build_nc(nc=None, loop_n=1):
    if nc is None:
        nc = bass.Bass()

    x8_d = nc.dram_tensor("x8", [NPER, CI, XSZ], F8E4, kind="ExternalInput")
    xq_d = nc.dram_tensor("xq", [CI, NPER, OH * OH], BF16, kind="ExternalInput")
    wyat_d = nc.dram_tensor("wyat8", [CI, 5, 2, CO], F8E4, kind="ExternalInput")
    wlin_d = nc.dram_tensor("wlin8", [CI, 2, 9, CO], F8E5, kind="ExternalInput")
    wshort_d = nc.dram_tensor("wshort", [CI, CO], BF16, kind="ExternalInput")
    wsqe_d = nc.dram_tensor("wsqe", [CI, 2], F32, kind="ExternalInput")
    out_d = nc.dram_tensor("out", [NPER, CI, 2, OH * OH], F32,
                           kind="ExternalOutput")

    with tile.TileContext(nc) as tc:
        with (
            tc.tile_pool(name="const", bufs=1) as const,
            tc.tile_pool(name="xsqp", bufs=2) as xsqp,
            tc.tile_pool(name="scr", bufs=3) as scr,
            tc.tile_pool(name="outp", bufs=2) as outp,
            tc.tile_pool(name="psum", bufs=8, space="PSUM") as psum,
        ):
            wyat_sb = const.tile([CI, 5, 2, CO], F8E4, tag="wyat")
            wlin_sb = const.tile([CI, 2, 9, CO], F8E5, tag="wlin")
            wshort_sb = const.tile([CI, CO], BF16, tag="wshort")
            wsqe_sb = const.tile([CI, 2], F32, tag="wsqe")
            ones_sb = const.tile([CI, 2, CI], F8E4, tag="ones")
            onez_sb = const.tile([CI, 2, CI], F8E4, tag="onez")
            ones1_sb = const.tile([1, 2 * NPIX], F8E4, tag="ones1")
            x8_sb = [const.tile([CI, XSZ], F8E4, tag=f"x{s}", name=f"x8_{s}")
                     for s in range(NPER)]
            xq_sb = const.tile([CI, NPER, OH * OH], BF16, tag="xq")
            yat_sb = [const.tile([CI, 2, YSZ], F8E4, tag=f"yat{s}",
                                 name=f"yat_{s}") for s in range(NPER)]

            def emit_iter(_it=0):
                XS = {}   # per-sample xsq tile
                PT = {}   # per-sample dot psums [c][t]
                PP = {}   # per-sample patch_sq psums [c]

                # head: constants + first loads
                nc.vector.memset(ones_sb[:], 1.0)
                nc.vector.memset(ones1_sb[:], 1.0)
                nc.gpsimd.memset(onez_sb[:, 0], 1.0)
                nc.gpsimd.memset(onez_sb[:, 1], 0.0)
                if _it == 0:
                    # prime the ACT Square table (~1.3us) during the idle
                    # head; square(1)=1 keeps the ones tile intact
                    nc.scalar.square(ones1_sb[:, :1], ones1_sb[:, :1])
                nc.gpsimd.dma_start(out=wyat_sb[:], in_=wyat_d[:])
                nc.sync.dma_start(out=x8_sb[0][:, :HALF0],
                                  in_=x8_d[0, :, :HALF0])
                nc.sync.dma_start(out=x8_sb[0][:, HALF0:],
                                  in_=x8_d[0, :, HALF0:])
                nc.sync.dma_start(out=wsqe_sb[:], in_=wsqe_d[:])
                if _it == 0:
                    # keep the PE p-state ramp warm through the DMA head
                    pw = psum.tile([CI, PBANK], F32, tag="ps", name="pwarm")
                    ones_flat = ones_sb[:].rearrange("p a b -> p (a b)")
                    for _w in range(WARMUP_MMS):
                        nc.tensor.matmul(pw[:, :2 * CI], ones_sb[:, 0],
                                         ones_flat, start=True, stop=True)
                nc.sync.dma_start(out=x8_sb[1][:], in_=x8_d[1])

                def prep(s):
                    """xsq = x^2 in e4m3. Sample 0 splits the two row-halves
                    across ACT and Pool so both chunks' patch_sq can start
                    as early as possible; padding squares to 0."""
                    xsq = xsqp.tile([CI, XSZ], F8E4, tag="xsq",
                                    name=f"xsq{s}")
                    XS[s] = xsq
                    sq = mybir.ActivationFunctionType.Square
                    if s == 0:
                        nc.scalar.activation(xsq[:, :HALF0],
                                             x8_sb[s][:, :HALF0], sq)
                        nc.gpsimd.tensor_mul(out=xsq[:, HALF0:],
                                             in0=x8_sb[s][:, HALF0:],
                                             in1=x8_sb[s][:, HALF0:])
                    else:
                        nc.scalar.activation(xsq[:], x8_sb[s][:], sq)
                    # yat plane pad rows (top 2, bottom 2)
                    for t in range(2):
                        nc.gpsimd.memset(yat_sb[s][:, t, :2 * OH], 0.0)
                        nc.gpsimd.memset(yat_sb[s][:, t, 30 * OH:YSZ], 0.0)

                def dp(s):
                    """conv1 dot + patch_sq matmuls."""
                    xflat = x8_sb[s][:]
                    qflat = XS[s][:]
                    part = xflat.ap[0]
                    qpart = qflat.ap[0]
                    pt = [[None, None], [None, None]]
                    pp = [None, None]
                    PT[s], PP[s] = pt, pp
                    for c in range(NCH):
                        for t in range(2):
                            ptile = psum.tile([CI, PBANK], F32, tag="ps",
                                              name=f"pt{s}_{c}_{t}")
                            pt[c][t] = ptile
                            for pi, (ta, tb) in enumerate(TAP_PAIRS):
                                oa = _xoff(ta // 3, ta % 3, c)
                                ob = (oa + 1 if tb is None
                                      else _xoff(tb // 3, tb % 3, c))
                                rhs = _rhs3(xflat, oa, part, ob - oa, NPIX)
                                lhsT = wyat_sb[:, pi, :, t * CI:(t + 1) * CI]
                                nc.tensor.matmul(
                                    ptile[:, :NPIX], lhsT, rhs, perf_mode=DR,
                                    start=(pi == 0), stop=(pi == 4))
                        # patch_sq: shared across co-tiles
                        pptile = psum.tile([CI, PBANK], F32, tag="ps",
                                           name=f"pp{s}_{c}")
                        pp[c] = pptile
                        for pi, (ta, tb) in enumerate(TAP_PAIRS):
                            oa = _xoff(ta // 3, ta % 3, c)
                            ob = (oa + 1 if tb is None
                                  else _xoff(tb // 3, tb % 3, c))
                            rhs = _rhs3(qflat, oa, qpart, ob - oa, NPIX)
                            lhsT = (onez_sb if tb is None else ones_sb)[:]
                            nc.tensor.matmul(
                                pptile[:, :NPIX], lhsT, rhs, perf_mode=DR,
                                start=(pi == 0), stop=(pi == 4))

                def chain(s):
                    """YAT elementwise: psqe -> d -> 1/d -> num -> yat8."""
                    pt, pp = PT[s], PP[s]
                    for c in range(NCH):
                        for t in range(2):
                            pq = scr.tile([CI, NPIX], F32, tag="q")
                            d4 = scr.tile([CI, NPIX], F32, tag="d")
                            r4 = scr.tile([CI, NPIX], F32, tag="r")
                            num = scr.tile([CI, NPIX], F32, tag="n")
                            nc.gpsimd.tensor_scalar(
                                out=pq[:], in0=pp[c][:, :NPIX],
                                scalar1=wsqe_sb[:, t:t + 1], scalar2=None,
                                op0=mybir.AluOpType.add)
                            nc.vector.scalar_tensor_tensor(
                                out=d4[:], in0=pt[c][t][:, :NPIX],
                                scalar=-0.125, in1=pq[:],
                                op0=mybir.AluOpType.mult,
                                op1=mybir.AluOpType.add)
                            nc.vector.reciprocal_approx_fast(out=r4[:],
                                                             in_=d4[:])
                            nc.gpsimd.tensor_mul(out=num[:],
                                                 in0=pt[c][t][:, :NPIX],
                                                 in1=pt[c][t][:, :NPIX])
                            nc.gpsimd.tensor_mul(
                                out=yat_sb[s][:, t,
                                              2 * OH + c * NPIX:
                                              2 * OH + (c + 1) * NPIX],
                                in0=num[:], in1=r4[:])

                def phase_b(s):
                    """conv2 (3x3 s1 p1, fp8 DR over ci pairs, wrap-corrected)
                    + bf16 1x1 s2 shortcut -> out."""
                    yflat = yat_sb[s][:].rearrange("p a b -> p (a b)")
                    ypart = yflat.ap[0]
                    last = s == NPER - 1
                    out_t = outp.tile([CI, 4 * NPIX], F32, tag="out")
                    for t in range(2):
                        for c in range(NCH):
                            po = psum.tile([CI, PBANK], F32, tag="ps",
                                           name=f"po{s}_{t}_{c}")
                            po3 = po[:, :NPIX].rearrange(
                                "p (r q) -> p r q", q=OH)
                            nc.tensor.matmul(
                                po[:, :NPIX],
                                wshort_sb[:, t * CI:(t + 1) * CI],
                                xq_sb[:, s, c * NPIX:(c + 1) * NPIX],
                                start=True, stop=False)
                            for ti in range(9):
                                kh, kw = ti // 3, ti % 3
                                off = (c * CH + kh + 1) * OH + kw - 1
                                rhs = _rhs3(yflat, off, ypart, YSZ, NPIX)
                                lhsT = wlin_sb[:, :, ti, t * CI:(t + 1) * CI]
                                nc.tensor.matmul(
                                    po[:, :NPIX], lhsT, rhs,
                                    perf_mode=DR, start=False, stop=False)
                            # accumulate the column-wrap spurious terms into
                            # the same bank's scratch region (contiguous psum
                            # writes only), then subtract into the edge cols
                            for ci_, ti in enumerate(COR_TAPS):
                                kh, kw = ti // 3, ti % 3
                                if kw == 0:
                                    off = (c * CH + kh) * OH + OH - 1
                                    oview = po[:, NPIX:NPIX + CH]
                                else:
                                    off = (c * CH + kh + 2) * OH
                                    oview = po[:, NPIX + CH:NPIX + 2 * CH]
                                rhs = _rhs3(yflat, off, ypart, YSZ, CH,
                                            nstride=OH)
                                lhsT = wlin_sb[:, :, ti,
                                               t * CI:(t + 1) * CI]
                                nc.tensor.matmul(
                                    oview, lhsT, rhs, perf_mode=DR,
                                    start=False, stop=(ci_ == 5))
                            eng = nc.vector if t == 0 else nc.gpsimd
                            sp0 = po[:, NPIX:NPIX + CH].rearrange(
                                "p (a b) -> p a b", b=1)
                            sp1 = po[:, NPIX + CH:NPIX + 2 * CH].rearrange(
                                "p (a b) -> p a b", b=1)
                            eng.tensor_sub(out=po3[:, :, 0:1],
                                           in0=po3[:, :, 0:1], in1=sp0)
                            eng.tensor_sub(out=po3[:, :, OH - 1:OH],
                                           in0=po3[:, :, OH - 1:OH], in1=sp1)
                            base = t * 2 * NPIX + c * NPIX
                            dst = out_t[:, base:base + NPIX]
                            if t == 0:
                                nc.scalar.copy(dst, po[:, :NPIX])
                            else:
                                nc.gpsimd.tensor_copy(out=dst, in_=po[:, :NPIX])
                            if last:
                                # spread the tail stores across HWDGE queues
                                q = [nc.sync, nc.scalar,
                                     nc.sync, nc.scalar][t * 2 + c]
                                q.dma_start(
                                    out=out_d[s, :, t,
                                              c * NPIX:(c + 1) * NPIX],
                                    in_=dst)
                    if not last:
                        nc.sync.dma_start(
                            out=out_d[s].rearrange("c t x -> c (t x)"),
                            in_=out_t[:])

                # software pipeline: PE order dp0,dp1,B0,dp2,B1,dp3,B2,B3;
                # ACT order sq0,sq1,sq2,cp0,sq3,cp1,cp2,cp3 (squares are on
                # the PE-feed path and must not queue behind output copies)
                prep(0)
                dp(0)
                chain(0)
                nc.sync.dma_start(out=x8_sb[2][:], in_=x8_d[2])
                nc.gpsimd.dma_start(out=wlin_sb[:], in_=wlin_d[:])
                nc.gpsimd.dma_start(out=wshort_sb[:], in_=wshort_d[:])
                prep(1)
                dp(1)
                chain(1)
                nc.sync.dma_start(out=x8_sb[3][:], in_=x8_d[3])
                nc.sync.dma_start(out=xq_sb[:], in_=xq_d[:])
                prep(2)
                phase_b(0)
                dp(2)
                chain(2)
                prep(3)
                phase_b(1)
                dp(3)
                chain(3)
                phase_b(2)
                phase_b(3)

            for _it in range(loop_n):
                emit_iter(_it)

    return nc


_NC_CACHE = {}


def _get_nc(loop_n=1):
    key = loop_n
    if key not in _NC_CACHE:
        nc = bacc.Bacc(None, target_bir_lowering=False)
        build_nc(nc=nc, loop_n=loop_n)
        nc.compile()
        _NC_CACHE[key] = nc
    return _NC_CACHE[key]


def out_to_full(arr):
    """[NPER, CI, 2, OH*OH] device layout -> [NPER, CO, OH, OH]."""
    return np.ascontiguousarray(arr.transpose(0, 2, 1, 3)).reshape(
        arr.shape[0], CO, OH, OH)


def prep_inputs(x, w_yat, alpha, w_lin, w_short):
    """Host-side dtype/layout prep for the full batch."""
    x = np.asarray(x, np.float32)
    w_yat = np.asarray(w_yat, np.float32)
    w_lin = np.asarray(w_lin, np.float32)
    w_short = np.asarray(w_short, np.float32)
    n = x.shape[0]

    # six 29x28 parity sub-planes with padding baked in:
    # plane (hh, kw): rows = padded rows hh,hh+2,..,hh+56; cols per kw:
    # kw=0: padded cols 0,2,..,54; kw=1: 1,3,..,55; kw=2: 2,4,..,56;
    # then row-split into rows 0..15 and rows 14..28 halves
    xpad = np.zeros((n, CI, 58, 58), np.float32)
    xpad[:, :, 1:H + 1, 1:H + 1] = x
    planes = np.empty((n, CI, 6, 29, OH), np.float32)
    for hh in range(2):
        rows = xpad[:, :, hh:hh + 58:2, :]
        planes[:, :, hh * 3 + 0] = rows[:, :, :, 0:56:2]
        planes[:, :, hh * 3 + 1] = rows[:, :, :, 1:57:2]
        planes[:, :, hh * 3 + 2] = rows[:, :, :, 2:58:2]
    p8 = planes.astype(E4)
    x8 = np.concatenate(
        [p8[:, :, :, 0:16, :].reshape(n, CI, HALF0),
         p8[:, :, :, 14:29, :].reshape(n, CI, HALF1)], axis=2)
    # xq: [CI, NPER-per-core..., pix] so all samples ride one DMA per core
    xq = np.ascontiguousarray(
        x[:, :, ::2, ::2].reshape(n, CI, OH * OH).transpose(1, 0, 2)
    ).astype(BF)

    # conv1 weights: x16, tap-paired [ci, pair, 2, co]
    wt = np.ascontiguousarray(
        (w_yat * np.float32(16.0)).transpose(1, 2, 3, 0)).reshape(CI, 9, CO)
    wyat8 = np.zeros((CI, 5, 2, CO), E4)
    wyat8[:, :4] = wt[:, :8].reshape(CI, 4, 2, CO).astype(E4)
    wyat8[:, 4, 0] = wt[:, 8].astype(E4)

    scale = float((np.sqrt(np.float32(CO)) / np.log1p(np.float32(CO)))
                  ** np.float32(np.asarray(alpha).ravel()[0]))
    wlin_t = np.ascontiguousarray(
        (w_lin * np.float32(scale / 256.0)).transpose(1, 2, 3, 0)
    ).reshape(2, CI, 9, CO).transpose(1, 0, 2, 3)
    wlin8 = np.ascontiguousarray(wlin_t).astype(E5)  # [ci, ci_tile, tap, co]

    wshort = np.ascontiguousarray(
        w_short[:, :, 0, 0].transpose(1, 0)).astype(BF)

    wsq = (w_yat.astype(np.float32) ** 2).sum(axis=(1, 2, 3))
    wsqe = np.ascontiguousarray(
        (wsq + np.float32(EPS)).reshape(2, CI).T).astype(np.float32)

    return {"x8": x8, "xq": xq, "wyat8": wyat8, "wlin8": wlin8,
            "wshort": wshort, "wsqe": wsqe}


def kernel(x, w_yat, alpha, w_lin, w_short, _trace=False):
    import os
    # this axon deployment has no NTFF hook (antenv.axon_hooks absent);
    # make sure an inherited BASS_TRACE can't route us into that path
    if not _trace:
        os.environ["BASS_NEVER_TRACE"] = "1"
    full = prep_inputs(x, w_yat, alpha, w_lin, w_short)
    nc = _get_nc()
    in_maps = []
    for i in range(N_CORES):
        m = {k: v for k, v in full.items() if k not in ("x8", "xq")}
        m["x8"] = full["x8"][i * NPER:(i + 1) * NPER]
        m["xq"] = np.ascontiguousarray(
            full["xq"][:, i * NPER:(i + 1) * NPER])
        in_maps.append(m)
    res = run_bass_kernel_spmd(nc, in_maps, core_ids=list(range(N_CORES)),
                               trace=_trace)
    out = np.concatenate([out_to_full(res.results[i]["out"])
                          for i in range(N_CORES)], axis=0)
    if _trace:
        kernel.last_results = res
    return out


# revision 13
# speedup vs baseline: 2.3595x; 1.0249x over previous
"""BasicYATBlock kernel for Trainium2 (Bass/Tile), data-parallel over batch on 8 cores.

Computes, per sample (stride=2 block, 128ch 56x56 -> 256ch 28x28):
    identity = conv1x1_s2(x, w_short)
    dot      = conv3x3_s2_p1(x, w_yat)
    patch_sq = conv3x3_s2_p1(x*x, ones)          (per-patch squared norm)
    yat      = dot^2 / (patch_sq + |w|^2 - 2 dot + EPS) * scale
    out      = conv3x3_s1_p1(yat, w_lin) + identity

All three convs are TensorE matmuls. The two big convs run in fp8 with
DoubleRow perf mode (2 K-tiles per instruction at 0.5 PE cycles/row).
DoubleRow ifmaps must be [p][2][N] access patterns, so layouts are built
to make every tap window one contiguous 392-element run:
  - x is uploaded as six 29x28 parity sub-planes (row parity hh x three
    column alignments kw=0/1/2, zero padding baked in on the host), row-
    split into a rows 0..15 half and a rows 14..28 half so the two output
    chunks' squares can start independently. A stride-2 3x3 tap is then
    rows a0..a0+13 x all 28 cols = one flat [off, off+392) run.
    conv1 pairs 2 taps per matmul (pair 5 = tap9 + zero weights).
  - patch_sq: same tap pairs over xsq = x^2 (squared fp8 planes), shared
    across both co-tiles; |w|^2+eps is added per co-tile by a cheap Pool
    tensor_scalar into SBUF (psqe) before the DVE chain.
  - conv2 DoubleRow-pairs the two ci tiles (K=256 per instruction) on a
    32x28 row-padded yat plane; kw!=1 taps read one column out of line,
    which wraps to the adjacent row's edge column — those spurious terms
    are accumulated by 6 tiny N=14 correction matmuls into scratch at the
    end of the same PSUM bank and subtracted from the edge columns.
The identity shortcut stays bf16 (it dominates the output magnitude;
fp8 there would eat most of the 2e-2 error budget). Scale bookkeeping:
x8=e4m3(x), wyat8=e4m3(16 w), xsq=e4m3(x^2), wsqe=f32(|w|^2+eps),
d = (pt * -1/8) + psqe = dist^2+eps; num = pt^2 = 256 dot^2;
yat8 = e4m3(256 yat); wlin8 = e5m2(w_lin*alpha_scale/256).
The 1/256 keeps wlin roughly in e5m2 normal range (reaches 6e-5) and
the 256 keeps yat8 in e4m3 normal range.

Elementwise work is spread so no engine exceeds the PE: ACT does the
squares (chunk-1 half of sample 0 goes to Pool so both chunks' patch_sq
can start early) + t0 output copies, DVE does d/recip, Pool (gpsimd)
does psqe/num/yat multiplies + t1 output copies. Output rides per-sample
[ci][co-tile][pix] DMAs (host transposes back); the last sample's DMAs
are split per chunk and spread across queues to shorten the tail.
"""

import numpy as np
import ml_dtypes

import bass_rust
import concourse.bass as bass
import concourse.bacc as bacc
import concourse.mybir as mybir
from concourse import tile
from concourse.bass_utils import run_bass_kernel_spmd

F32 = mybir.dt.float32
BF16 = mybir.dt.bfloat16
F8E4 = mybir.dt.float8e4
F8E5 = mybir.dt.float8e5

E4 = ml_dtypes.float8_e4m3
E5 = ml_dtypes.float8_e5m2
BF = ml_dtypes.bfloat16

N_CORES = 8
NPER = 4          # samples per core
CI = 128          # input channels
CO = 256          # output channels (2 tiles of 128)
H = 56            # input spatial
OH = 28           # output spatial
CH = 14           # output rows per chunk
NCH = 2           # chunks per plane
NPIX = CH * OH    # 392 free elements per PSUM tile
PBANK = NPIX + 2 * CH  # psum slot: 392 out + 2x14 wrap-correction scratch
HALF0 = 6 * 16 * OH    # 2688: rows 0..15 of the six sub-planes
HALF1 = 6 * 15 * OH    # 2520: rows 14..28
XSZ = HALF0 + HALF1    # 5208 per sample
YROWS = 32        # yat plane rows (2 top pad + 28 + 2 bottom pad)
YSZ = YROWS * OH  # 896
EPS = 0.007

# conv1/patch_sq tap pairs (tap index = kh*3+kw); pair 5 = (tap 8, dummy)
TAP_PAIRS = [(0, 1), (2, 3), (4, 5), (6, 7), (8, None)]
# conv2 correction taps: kw=0 group then kw=2 group
COR_TAPS = [0, 3, 6, 2, 5, 8]
WARMUP_MMS = 10
DR = mybir.MatmulPerfMode.DoubleRow


def _xoff(kh, kw, c):
    """Flat offset of stride-2 tap (kh,kw), chunk c in the half-split
    6-sub-plane x layout."""
    pl = (kh % 2) * 3 + kw
    a0 = c * CH + (1 if kh == 2 else 0)
    if c == 0:
        return pl * (16 * OH) + a0 * OH
    return HALF0 + pl * (15 * OH) + (a0 - CH) * OH


def _rhs3(flat_ap, off, part_dim, pair_stride, n, nstride=1):
    """3-dim DoubleRow rhs AP [p][pair 2][n]."""
    d = flat_ap.copy()
    d.ap = bass_rust.VecI64Pair([list(part_dim), [pair_stride, 2],
                                 [nstride, n]])
    d.offset = flat_ap.offset + off
    return d


def build_nc(nc=None, loop_n=1):
    if nc is None:
        nc = bass.Bass()

    x8_d = nc.dram_tensor("x8", [NPER, CI, XSZ], F8E4, kind="ExternalInput")
    xq_d = nc.dram_tensor("xq", [CI, NPER, OH * OH], BF16, kind="ExternalInput")
    wyat_d = nc.dram_tensor("wyat8", [CI, 5, 2, CO], F8E4, kind="ExternalInput")
    wlin_d = nc.dram_tensor("wlin8", [CI, 2, 9, CO], F8E5, kind="ExternalInput")
    wshort_d = nc.dram_tensor("wshort", [CI, CO], BF16, kind="ExternalInput")
    wsqe_d = nc.dram_tensor("wsqe", [CI, 2], F32, kind="ExternalInput")
    out_d = nc.dram_tensor("out", [NPER, CI, 2, OH * OH], F32,
                           kind="ExternalOutput")

    with tile.TileContext(nc) as tc:
        with (
            tc.tile_pool(name="const", bufs=1) as const,
            tc.tile_pool(name="xsqp", bufs=2) as xsqp,
            tc.tile_pool(name="scr", bufs=3) as scr,
            tc.tile_pool(name="outp", bufs=2) as outp,
            tc.tile_pool(name="psum", bufs=8, space="PSUM") as psum,
        ):
            wyat_sb = const.tile([CI, 5, 2, CO], F8E4, tag="wyat")
            wlin_sb = const.tile([CI, 2, 9, CO], F8E5, tag="wlin")
            wshort_sb = const.tile([CI, CO], BF16, tag="wshort")
            wsqe_sb = const.tile([CI, 2], F32, tag="wsqe")
            ones_sb = const.tile([CI, 2, CI], F8E4, tag="ones")
            onez_sb = const.tile([CI, 2, CI], F8E4, tag="onez")
            ones1_sb = const.tile([1, 2 * NPIX], F8E4, tag="ones1")
            x8_sb = [const.tile([CI, XSZ], F8E4, tag=f"x{s}", name=f"x8_{s}")
                     for s in range(NPER)]
            xq_sb = const.tile([CI, NPER, OH * OH], BF16, tag="xq")
            yat_sb = [const.tile([CI, 2, YSZ], F8E4, tag=f"yat{s}",
                                 name=f"yat_{s}") for s in range(NPER)]

            def emit_iter(_it=0):
                XS = {}   # per-sample xsq tile
                PT = {}   # per-sample dot psums [c][t]
                PP = {}   # per-sample patch_sq psums [c]

                # head: constants + first loads
                nc.vector.memset(ones1_sb[:, :1], 1.0)
                nc.vector.memset(ones_sb[:], 1.0)
                nc.gpsimd.memset(ones1_sb[:, 1:], 1.0)
                nc.gpsimd.memset(onez_sb[:, 0], 1.0)
                nc.gpsimd.memset(onez_sb[:, 1], 0.0)
                if _it == 0:
                    # prime the ACT Square table (~1.3us) during the idle
                    # head; square(1)=1 keeps the ones tile intact
                    nc.scalar.square(ones1_sb[:, :1], ones1_sb[:, :1])
                nc.gpsimd.dma_start(out=wyat_sb[:], in_=wyat_d[:])
                nc.sync.dma_start(out=x8_sb[0][:, :HALF0],
                                  in_=x8_d[0, :, :HALF0])
                nc.sync.dma_start(out=x8_sb[0][:, HALF0:],
                                  in_=x8_d[0, :, HALF0:])
                nc.sync.dma_start(out=x8_sb[1][:], in_=x8_d[1])
                nc.sync.dma_start(out=wsqe_sb[:], in_=wsqe_d[:])
                if _it == 0:
                    # keep the PE p-state ramp warm through the DMA head
                    pw = psum.tile([CI, PBANK], F32, tag="ps", name="pwarm")
                    ones_flat = ones_sb[:].rearrange("p a b -> p (a b)")
                    for _w in range(WARMUP_MMS):
                        nc.tensor.matmul(pw[:, :2 * CI], ones_sb[:, 0],
                                         ones_flat, start=True, stop=True)

                def prep(s):
                    """xsq = x^2 in e4m3. Sample 0 splits the two row-halves
                    across ACT and Pool so both chunks' patch_sq can start
                    as early as possible; padding squares to 0."""
                    xsq = xsqp.tile([CI, XSZ], F8E4, tag="xsq",
                                    name=f"xsq{s}")
                    XS[s] = xsq
                    sq = mybir.ActivationFunctionType.Square
                    if s <= 1:
                        nc.scalar.activation(xsq[:, :HALF0],
                                             x8_sb[s][:, :HALF0], sq)
                        nc.gpsimd.tensor_mul(out=xsq[:, HALF0:],
                                             in0=x8_sb[s][:, HALF0:],
                                             in1=x8_sb[s][:, HALF0:])
                    else:
                        nc.scalar.activation(xsq[:], x8_sb[s][:], sq)
                    # yat plane pad rows (top 2, bottom 2)
                    for t in range(2):
                        nc.gpsimd.memset(yat_sb[s][:, t, :2 * OH], 0.0)
                        nc.gpsimd.memset(yat_sb[s][:, t, 30 * OH:YSZ], 0.0)

                def dp(s):
                    """conv1 dot + patch_sq matmuls."""
                    xflat = x8_sb[s][:]
                    qflat = XS[s][:]
                    part = xflat.ap[0]
                    qpart = qflat.ap[0]
                    pt = [[None, None], [None, None]]
                    pp = [None, None]
                    PT[s], PP[s] = pt, pp
                    for c in range(NCH):
                        for t in range(2):
                            ptile = psum.tile([CI, PBANK], F32, tag="ps",
                                              name=f"pt{s}_{c}_{t}")
                            pt[c][t] = ptile
                            for pi, (ta, tb) in enumerate(TAP_PAIRS):
                                oa = _xoff(ta // 3, ta % 3, c)
                                ob = (oa + 1 if tb is None
                                      else _xoff(tb // 3, tb % 3, c))
                                rhs = _rhs3(xflat, oa, part, ob - oa, NPIX)
                                lhsT = wyat_sb[:, pi, :, t * CI:(t + 1) * CI]
                                nc.tensor.matmul(
                                    ptile[:, :NPIX], lhsT, rhs, perf_mode=DR,
                                    start=(pi == 0), stop=(pi == 4))
                    for c in range(NCH):
                        # patch_sq: shared across co-tiles
                        pptile = psum.tile([CI, PBANK], F32, tag="ps",
                                           name=f"pp{s}_{c}")
                        pp[c] = pptile
                        for pi, (ta, tb) in enumerate(TAP_PAIRS):
                            oa = _xoff(ta // 3, ta % 3, c)
                            ob = (oa + 1 if tb is None
                                  else _xoff(tb // 3, tb % 3, c))
                            rhs = _rhs3(qflat, oa, qpart, ob - oa, NPIX)
                            lhsT = (onez_sb if tb is None else ones_sb)[:]
                            nc.tensor.matmul(
                                pptile[:, :NPIX], lhsT, rhs, perf_mode=DR,
                                start=(pi == 0), stop=(pi == 4))

                def chain(s):
                    """YAT elementwise: psqe -> d -> 1/d -> num -> yat8."""
                    pt, pp = PT[s], PP[s]
                    for c in range(NCH):
                        for t in range(2):
                            pq = scr.tile([CI, NPIX], F32, tag="q")
                            d4 = scr.tile([CI, NPIX], F32, tag="d")
                            r4 = scr.tile([CI, NPIX], F32, tag="r")
                            num = scr.tile([CI, NPIX], F32, tag="n")
                            nc.gpsimd.tensor_scalar(
                                out=pq[:], in0=pp[c][:, :NPIX],
                                scalar1=wsqe_sb[:, t:t + 1], scalar2=None,
                                op0=mybir.AluOpType.add)
                            nc.vector.scalar_tensor_tensor(
                                out=d4[:], in0=pt[c][t][:, :NPIX],
                                scalar=-0.125, in1=pq[:],
                                op0=mybir.AluOpType.mult,
                                op1=mybir.AluOpType.add)
                            nc.vector.reciprocal_approx_fast(out=r4[:],
                                                             in_=d4[:])
                            num_eng = nc.vector if t == 0 else nc.gpsimd
                            num_eng.tensor_mul(out=num[:],
                                               in0=pt[c][t][:, :NPIX],
                                               in1=pt[c][t][:, :NPIX])
                            nc.gpsimd.tensor_mul(
                                out=yat_sb[s][:, t,
                                              2 * OH + c * NPIX:
                                              2 * OH + (c + 1) * NPIX],
                                in0=num[:], in1=r4[:])

                def phase_b(s):
                    """conv2 (3x3 s1 p1, fp8 DR over ci pairs, wrap-corrected)
                    + bf16 1x1 s2 shortcut -> out."""
                    yflat = yat_sb[s][:].rearrange("p a b -> p (a b)")
                    ypart = yflat.ap[0]
                    last = s == NPER - 1
                    out_t = outp.tile([CI, 4 * NPIX], F32, tag="out")
                    for t in range(2):
                        for c in range(NCH):
                            po = psum.tile([CI, PBANK], F32, tag="ps",
                                           name=f"po{s}_{t}_{c}")
                            po3 = po[:, :NPIX].rearrange(
                                "p (r q) -> p r q", q=OH)
                            nc.tensor.matmul(
                                po[:, :NPIX],
                                wshort_sb[:, t * CI:(t + 1) * CI],
                                xq_sb[:, s, c * NPIX:(c + 1) * NPIX],
                                start=True, stop=False)
                            for ti in range(9):
                                kh, kw = ti // 3, ti % 3
                                off = (c * CH + kh + 1) * OH + kw - 1
                                rhs = _rhs3(yflat, off, ypart, YSZ, NPIX)
                                lhsT = wlin_sb[:, :, ti, t * CI:(t + 1) * CI]
                                nc.tensor.matmul(
                                    po[:, :NPIX], lhsT, rhs,
                                    perf_mode=DR, start=False, stop=False)
                            # accumulate the column-wrap spurious terms into
                            # the same bank's scratch region (contiguous psum
                            # writes only), then subtract into the edge cols
                            for ci_, ti in enumerate(COR_TAPS):
                                kh, kw = ti // 3, ti % 3
                                if kw == 0:
                                    off = (c * CH + kh) * OH + OH - 1
                                    oview = po[:, NPIX:NPIX + CH]
                                else:
                                    off = (c * CH + kh + 2) * OH
                                    oview = po[:, NPIX + CH:NPIX + 2 * CH]
                                rhs = _rhs3(yflat, off, ypart, YSZ, CH,
                                            nstride=OH)
                                lhsT = wlin_sb[:, :, ti,
                                               t * CI:(t + 1) * CI]
                                nc.tensor.matmul(
                                    oview, lhsT, rhs, perf_mode=DR,
                                    start=False, stop=(ci_ == 5))
                            eng = nc.gpsimd
                            sp0 = po[:, NPIX:NPIX + CH].rearrange(
                                "p (a b) -> p a b", b=1)
                            sp1 = po[:, NPIX + CH:NPIX + 2 * CH].rearrange(
                                "p (a b) -> p a b", b=1)
                            eng.tensor_sub(out=po3[:, :, 0:1],
                                           in0=po3[:, :, 0:1], in1=sp0)
                            eng.tensor_sub(out=po3[:, :, OH - 1:OH],
                                           in0=po3[:, :, OH - 1:OH], in1=sp1)
                            base = t * 2 * NPIX + c * NPIX
                            dst = out_t[:, base:base + NPIX]
                            if t == 0:
                                nc.scalar.copy(dst, po[:, :NPIX])
                            else:
                                nc.gpsimd.tensor_copy(out=dst, in_=po[:, :NPIX])
                            if last:
                                # spread the tail stores across HWDGE queues
                                q = [nc.sync, nc.scalar,
                                     nc.sync, nc.scalar][t * 2 + c]
                                q.dma_start(
                                    out=out_d[s, :, t,
                                              c * NPIX:(c + 1) * NPIX],
                                    in_=dst)
                    if not last:
                        nc.sync.dma_start(
                            out=out_d[s].rearrange("c t x -> c (t x)"),
                            in_=out_t[:])

                # software pipeline: PE order dp0,dp1,B0,dp2,B1,dp3,B2,B3;
                # ACT order sq0,sq1,sq2,cp0,sq3,cp1,cp2,cp3 (squares are on
                # the PE-feed path and must not queue behind output copies)
                prep(0)
                dp(0)
                chain(0)
                nc.sync.dma_start(out=x8_sb[2][:], in_=x8_d[2])
                nc.gpsimd.dma_start(out=wlin_sb[:], in_=wlin_d[:])
                nc.gpsimd.dma_start(out=wshort_sb[:], in_=wshort_d[:])
                prep(1)
                dp(1)
                chain(1)
                nc.sync.dma_start(out=x8_sb[3][:], in_=x8_d[3])
                nc.sync.dma_start(out=xq_sb[:], in_=xq_d[:])
                prep(2)
                phase_b(0)
                dp(2)
                chain(2)
                prep(3)
                phase_b(1)
                dp(3)
                chain(3)
                phase_b(2)
                phase_b(3)

            for _it in range(loop_n):
                emit_iter(_it)

    return nc


_NC_CACHE = {}


def _get_nc(loop_n=1):
    key = loop_n
    if key not in _NC_CACHE:
        nc = bacc.Bacc(None, target_bir_lowering=False)
        build_nc(nc=nc, loop_n=loop_n)
        nc.compile()
        _NC_CACHE[key] = nc
    return _NC_CACHE[key]


def out_to_full(arr):
    """[NPER, CI, 2, OH*OH] device layout -> [NPER, CO, OH, OH]."""
    return np.ascontiguousarray(arr.transpose(0, 2, 1, 3)).reshape(
        arr.shape[0], CO, OH, OH)


def prep_inputs(x, w_yat, alpha, w_lin, w_short):
    """Host-side dtype/layout prep for the full batch."""
    x = np.asarray(x, np.float32)
    w_yat = np.asarray(w_yat, np.float32)
    w_lin = np.asarray(w_lin, np.float32)
    w_short = np.asarray(w_short, np.float32)
    n = x.shape[0]

    # six 29x28 parity sub-planes with padding baked in:
    # plane (hh, kw): rows = padded rows hh,hh+2,..,hh+56; cols per kw:
    # kw=0: padded cols 0,2,..,54; kw=1: 1,3,..,55; kw=2: 2,4,..,56;
    # then row-split into rows 0..15 and rows 14..28 halves
    xpad = np.zeros((n, CI, 58, 58), np.float32)
    xpad[:, :, 1:H + 1, 1:H + 1] = x
    planes = np.empty((n, CI, 6, 29, OH), np.float32)
    for hh in range(2):
        rows = xpad[:, :, hh:hh + 58:2, :]
        planes[:, :, hh * 3 + 0] = rows[:, :, :, 0:56:2]
        planes[:, :, hh * 3 + 1] = rows[:, :, :, 1:57:2]
        planes[:, :, hh * 3 + 2] = rows[:, :, :, 2:58:2]
    p8 = planes.astype(E4)
    x8 = np.concatenate(
        [p8[:, :, :, 0:16, :].reshape(n, CI, HALF0),
         p8[:, :, :, 14:29, :].reshape(n, CI, HALF1)], axis=2)
    # xq: [CI, NPER-per-core..., pix] so all samples ride one DMA per core
    xq = np.ascontiguousarray(
        x[:, :, ::2, ::2].reshape(n, CI, OH * OH).transpose(1, 0, 2)
    ).astype(BF)

    # conv1 weights: x16, tap-paired [ci, pair, 2, co]
    wt = np.ascontiguousarray(
        (w_yat * np.float32(16.0)).transpose(1, 2, 3, 0)).reshape(CI, 9, CO)
    wyat8 = np.zeros((CI, 5, 2, CO), E4)
    wyat8[:, :4] = wt[:, :8].reshape(CI, 4, 2, CO).astype(E4)
    wyat8[:, 4, 0] = wt[:, 8].astype(E4)

    scale = float((np.sqrt(np.float32(CO)) / np.log1p(np.float32(CO)))
                  ** np.float32(np.asarray(alpha).ravel()[0]))
    wlin_t = np.ascontiguousarray(
        (w_lin * np.float32(scale / 256.0)).transpose(1, 2, 3, 0)
    ).reshape(2, CI, 9, CO).transpose(1, 0, 2, 3)
    wlin8 = np.ascontiguousarray(wlin_t).astype(E5)  # [ci, ci_tile, tap, co]

    wshort = np.ascontiguousarray(
        w_short[:, :, 0, 0].transpose(1, 0)).astype(BF)

    wsq = (w_yat.astype(np.float32) ** 2).sum(axis=(1, 2, 3))
    wsqe = np.ascontiguousarray(
        (wsq + np.float32(EPS)).reshape(2, CI).T).astype(np.float32)

    return {"x8": x8, "xq": xq, "wyat8": wyat8, "wlin8": wlin8,
            "wshort": wshort, "wsqe": wsqe}


def kernel(x, w_yat, alpha, w_lin, w_short, _trace=False):
    import os
    # this axon deployment has no NTFF hook (antenv.axon_hooks absent);
    # make sure an inherited BASS_TRACE can't route us into that path
    if not _trace:
        os.environ["BASS_NEVER_TRACE"] = "1"
    full = prep_inputs(x, w_yat, alpha, w_lin, w_short)
    nc = _get_nc()
    in_maps = []
    for i in range(N_CORES):
        m = {k: v for k, v in full.items() if k not in ("x8", "xq")}
        m["x8"] = full["x8"][i * NPER:(i + 1) * NPER]
        m["xq"] = np.ascontiguousarray(
            full["xq"][:, i * NPER:(i + 1) * NPER])
        in_maps.append(m)
    res = run_bass_kernel_spmd(nc, in_maps, core_ids=list(range(N_CORES)),
                               trace=_trace)
    out = np.concatenate([out_to_full(res.results[i]["out"])
                          for i in range(N_CORES)], axis=0)
    if _trace:
        kernel.last_results = res
    return out
